# revision 12
# baseline (speedup 1.0000x reference)
"""DeepFM forward kernel for Trainium2, data-parallel over 8 NeuronCores.

Math refactor vs the straightforward DeepFM graph:
  sum_ij fm_interactions[b,i,j] = sum_k (sum_i m[b,i,k]) * (sum_j u[b,j,k])
so the BxNMxNU einsum collapses to an 18-dim per-row dot product of "folded"
tower outputs (16 fold products + the two additive terms via const-1 rows).

W1 is folded into the tower weights host-side (Wm1 = Wm_dense @ W1_top,
Wu1 = Wu_dense @ W1_bot) - legal because the first nonlinearity comes after
W1 - so the PE computes h1_pre directly from the inputs.

The logit is dominated by the FM interaction term (std ~64 vs ~1 for the
MLP output), so the whole h1/MLP path runs in fp8e4m3 with DoubleRow
packing (2 contraction rows per PE cell, 2x throughput): 8+1 double-passes
per tile replace 18 bf16 passes, at a negligible output-error cost. The
extras (FM) path stays bf16. fp8 operands are pre-scaled away from the
subnormal range; scales commute with relu and are undone in the
activations. Inputs ship as fp8 (h1) and bf16 (extras) in a handful of
large, consumption-ordered DMAs (DMA-completion semaphore lanes recycle
at ~2.5us each, so DMA count is a first-order cost).
"""

import numpy as np
import ml_dtypes

import concourse.bacc as bacc
import concourse.bass as bass  # noqa: F401
import concourse.mybir as mybir
import concourse.tile as tile
from concourse.bass_utils import run_bass_kernel_spmd

N_CORES = 8
B_FULL = 16384
R = B_FULL // N_CORES  # 2048 rows per core
F = 512                # input features per tower
KC = F // 128          # 4 contraction chunks per tower
NT = 512               # batch tile on the free dim
NTILES = R // NT       # 4
NX = 18                # fold(16) + [add_m, 1] / [1, add_u] rows
N_WARM = 8             # PE pre-warm matmuls (cover the DMA-receipt window)
SW = 16.0              # fp8 tower-weight pre-scale
SX = 4.0               # fp8 input pre-scale
SH = 4.0               # fp8 h1 activation pre-scale
SW2 = 16.0             # fp8 W2 pre-scale

F32 = mybir.dt.float32
BF16 = mybir.dt.bfloat16
FP8 = mybir.dt.float8e4
BF16_NP = ml_dtypes.bfloat16
FP8_NP = ml_dtypes.float8_e4m3
DR = mybir.MatmulPerfMode.DoubleRow

W8COLS = 4 * KC * 128 + 256     # 4 tower groups + W2 double-block

# bf16 weight-pack column offsets ([128, WCOLS] blob)
EXM_OFF = 0
EXU_OFF = EXM_OFF + KC * NX     # 72
W3_OFF = EXU_OFF + KC * NX      # 144
ONES_OFF = W3_OFF + 1           # 145
BIAS_OFF = ONES_OFF + 1         # 146
B1A, B1B, B2C, BMX, BUX = range(5)
WCOLS = BIAS_OFF + 5            # 151


def _fold_ext(W, b, is_movie, b3=0.0):
    """[512,257],[257] -> ([512,18], [18]) extras weights/bias."""
    dense_w = W[:, :256]
    fold_w = dense_w.reshape(F, 16, 16).sum(axis=1)        # [512, 16]
    add_w = W[:, 256:257]
    zero_w = np.zeros_like(add_w)
    fold_b = b[:256].reshape(16, 16).sum(axis=0)
    if is_movie:
        tail_w = [add_w, zero_w]
        tail_b = [b[256:257] + b3, np.ones(1, np.float32)]
    else:
        tail_w = [zero_w, add_w]
        tail_b = [np.ones(1, np.float32), b[256:257]]
    w_ext = np.concatenate([fold_w, *tail_w], axis=1)
    b_ext = np.concatenate([fold_b, *tail_b])
    return w_ext.astype(np.float32), b_ext.astype(np.float32)


def _chunk(Wext):
    """[K, M] -> [128, (K/128)*M]: K-chunk k occupies cols [k*M, (k+1)*M)."""
    kc, m = Wext.shape[0] // 128, Wext.shape[1]
    return Wext.reshape(kc, 128, m).transpose(1, 0, 2).reshape(128, kc * m)


def _col(vec):
    out = np.zeros((128, 1), np.float32)
    out[: len(vec), 0] = vec
    return out


def _pack_weights(Wm, bm, Wu, bu, W1, b1, W2, b2, W3, b3):
    b3v = float(np.asarray(b3, np.float32).reshape(1)[0])
    W1t, W1b = W1[:256], W1[256:]
    Wm1 = Wm[:, :256] @ W1t                        # [512, 256] fused tower+W1
    Wu1 = Wu[:, :256] @ W1b
    b1p = b1 + bm[:256] @ W1t + bu[:256] @ W1b     # [256]
    exm_w, exm_b = _fold_ext(Wm, bm, True, b3v)
    exu_w, exu_b = _fold_ext(Wu, bu, False)
    ones = np.zeros((128, 1), np.float32)
    ones[:NX, 0] = 1.0

    # fp8 DoubleRow pack: tower groups [128, g, k, m], then W2 [128, 2, 128]
    w8 = np.zeros((128, 4, KC, 128), np.float32)
    for g, Wg in enumerate((Wm1[:, :128], Wu1[:, :128], Wm1[:, 128:], Wu1[:, 128:])):
        w8[:, g] = (Wg * SW).reshape(KC, 128, 128).transpose(1, 0, 2)
    w2d = (W2.astype(np.float32) * SW2).reshape(2, 128, 128).transpose(1, 0, 2)
    wp8 = np.concatenate(
        [w8.reshape(128, 4 * KC * 128), w2d.reshape(128, 256)], axis=1
    )
    assert wp8.shape == (128, W8COLS)
    wp8 = np.ascontiguousarray(wp8.astype(FP8_NP))

    wp = np.concatenate(
        [
            _chunk(exm_w),
            _chunk(exu_w),
            W3.astype(np.float32).reshape(128, 1),
            ones,
            _col(b1p[:128] * SH), _col(b1p[128:] * SH), _col(b2.astype(np.float32)),
            _col(exm_b), _col(exu_b),
        ],
        axis=1,
    )
    assert wp.shape == (128, WCOLS), wp.shape
    return wp8, wp.astype(BF16_NP)


def _build_bass():
    nc = bacc.Bacc()
    xm8 = nc.dram_tensor("xm8", [128, NTILES * KC * NT], FP8, kind="ExternalInput")
    xu8 = nc.dram_tensor("xu8", [128, NTILES * KC * NT], FP8, kind="ExternalInput")
    xm = nc.dram_tensor("xm", [128, NTILES * KC * NT], BF16, kind="ExternalInput")
    xu = nc.dram_tensor("xu", [128, NTILES * KC * NT], BF16, kind="ExternalInput")
    wp8 = nc.dram_tensor("wp8", [128, W8COLS], FP8, kind="ExternalInput")
    wp = nc.dram_tensor("wp", [128, WCOLS], BF16, kind="ExternalInput")
    out = nc.dram_tensor("out", [1, R], F32, kind="ExternalOutput")

    relu = mybir.ActivationFunctionType.Relu
    add = mybir.AluOpType.add
    mult = mybir.AluOpType.mult

    with tile.TileContext(nc) as tc:
        with (
            tc.tile_pool(name="wpool", bufs=1) as wpool,
            tc.tile_pool(name="xpool", bufs=1) as xpool,
            tc.tile_pool(name="dpool", bufs=2) as dpool,
            tc.tile_pool(name="epool", bufs=2) as epool,
            tc.tile_pool(name="opool", bufs=1) as opool,
            tc.tile_pool(name="ps1", bufs=3, space="PSUM") as ps1,
            tc.tile_pool(name="psx", bufs=2, space="PSUM") as psx,
            tc.tile_pool(name="ps2", bufs=1, space="PSUM") as ps2p,
            tc.tile_pool(name="psf", bufs=1, space="PSUM") as psfp,
        ):
            # PE pre-warm on a zeroed tile: keeps the PE busy through the
            # initial DMA+receipt window so real matmuls start warm.
            wgar = wpool.tile([128, NT], BF16)
            nc.vector.memset(wgar, 0.0)
            for _ in range(N_WARM):
                pw = ps1.tile([128, NT], F32, name="ps_mm")
                nc.tensor.matmul(pw, wgar[:, :128], wgar, start=True, stop=True)

            wf8 = wpool.tile([128, 4 * KC + 2, 128], FP8)
            w = wpool.tile([128, WCOLS], BF16)
            b = wpool.tile([128, 5], F32)

            # Consumption-ordered DMAs: weights first (scalar), then
            # per-tile fp8 blocks interleaved with per-half bf16 blocks so
            # the PE surfs the ~400GB/s stream; outputs ride the SWDGE
            # ring so they don't occupy HWDGE semaphore lanes.
            nc.scalar.dma_start(
                out=wf8, in_=wp8.rearrange("p (c m) -> p c m", m=128)
            )
            nc.scalar.dma_start(out=w, in_=wp[:, :])
            nc.vector.tensor_copy(b, w[:, BIAS_OFF:])

            xm8_t = [xpool.tile([128, KC, NT], FP8, name=f"xm8_{t}") for t in range(NTILES)]
            xu8_t = [xpool.tile([128, KC, NT], FP8, name=f"xu8_{t}") for t in range(NTILES)]
            xm_h = [xpool.tile([128, 2, KC, NT], BF16, name=f"xmh{h}") for h in range(2)]
            xu_h = [xpool.tile([128, 2, KC, NT], BF16, name=f"xuh{h}") for h in range(2)]
            xm8r = xm8.rearrange("p (t k n) -> p t k n", t=NTILES, k=KC)
            xu8r = xu8.rearrange("p (t k n) -> p t k n", t=NTILES, k=KC)
            xmr = xm.rearrange("p (h t k n) -> p h t k n", h=2, t=2, k=KC)
            xur = xu.rearrange("p (h t k n) -> p h t k n", h=2, t=2, k=KC)
            for h in range(2):
                nc.sync.dma_start(out=xm8_t[2 * h], in_=xm8r[:, 2 * h])
                nc.scalar.dma_start(out=xu8_t[2 * h], in_=xu8r[:, 2 * h])
                nc.sync.dma_start(out=xm8_t[2 * h + 1], in_=xm8r[:, 2 * h + 1])
                nc.scalar.dma_start(out=xu8_t[2 * h + 1], in_=xu8r[:, 2 * h + 1])
                nc.sync.dma_start(out=xm_h[h], in_=xmr[:, h])
                nc.scalar.dma_start(out=xu_h[h], in_=xur[:, h])

            out_sb = opool.tile([1, R], F32)

            for t in range(NTILES):
                n0 = t * NT
                h, tt = divmod(t, 2)
                xm8t, xu8t = xm8_t[t], xu8_t[t]
                xmt, xut = xm_h[h][:, tt], xu_h[h][:, tt]

                # --- fused tower+W1 in fp8 DoubleRow: 2 halves x 4 passes ---
                h1p = dpool.tile([128, 2, NT], FP8, name="h1p")
                for half in range(2):
                    ps = ps1.tile([128, NT], F32, name="ps_mm")
                    for g, xt in ((2 * half, xm8t), (2 * half + 1, xu8t)):
                        for c in range(2):
                            nc.tensor.matmul(
                                ps, wf8[:, g * KC + 2 * c : g * KC + 2 * c + 2, :],
                                xt[:, 2 * c : 2 * c + 2, :],
                                start=(g == 2 * half and c == 0),
                                stop=(g == 2 * half + 1 and c == 1),
                                perf_mode=DR,
                            )
                    # h1' = relu(ps/(SW*SX) + b1)*SH, stored fp8
                    nc.scalar.activation(
                        out=h1p[:, half, :], in_=ps, func=relu,
                        bias=b[:, B1A + half : B1A + half + 1],
                        scale=SH / (SW * SX),
                    )

                # --- tower extras (bf16): two M=18 groups ---
                psxm = psx.tile([NX, NT], F32, name="ps_x")
                psxu = psx.tile([NX, NT], F32, name="ps_x")
                for k in range(KC):
                    lm = w[:, EXM_OFF + k * NX : EXM_OFF + (k + 1) * NX]
                    nc.tensor.matmul(psxm, lm, xmt[:, k, :],
                                     start=(k == 0), stop=(k == KC - 1))
                for k in range(KC):
                    lu = w[:, EXU_OFF + k * NX : EXU_OFF + (k + 1) * NX]
                    nc.tensor.matmul(psxu, lu, xut[:, k, :],
                                     start=(k == 0), stop=(k == KC - 1))
                dmx = epool.tile([NX, NT], BF16, name="dmx")
                nc.vector.tensor_scalar_add(out=dmx, in0=psxm,
                                            scalar1=b[:NX, BMX : BMX + 1])
                prod = epool.tile([NX, NT], BF16, name="prod")
                nc.vector.scalar_tensor_tensor(
                    out=prod, in0=psxu, scalar=b[:NX, BUX : BUX + 1], in1=dmx,
                    op0=add, op1=mult,
                )

                # --- MLP layer 2: one fp8 DoubleRow pass (K=256) ---
                ps2 = ps2p.tile([128, NT], F32, name="ps_l2")
                nc.tensor.matmul(ps2, wf8[:, 4 * KC : 4 * KC + 2, :], h1p,
                                 start=True, stop=True, perf_mode=DR)
                h2 = dpool.tile([128, NT], BF16, name="h2")
                nc.scalar.activation(out=h2, in_=ps2, func=relu,
                                     bias=b[:, B2C : B2C + 1],
                                     scale=1.0 / (SH * SW2))

                # --- final: logit = W3.T @ h2 + ones18.T @ prod ---
                psf = psfp.tile([1, NT], F32, name="ps_fin")
                nc.tensor.matmul(psf, w[:, W3_OFF : W3_OFF + 1], h2,
                                 start=True, stop=False)
                nc.tensor.matmul(psf, w[:NX, ONES_OFF : ONES_OFF + 1], prod,
                                 start=False, stop=True)
                nc.vector.tensor_copy(out_sb[:, n0 : n0 + NT], psf)
                nc.gpsimd.dma_start(out=out[:, n0 : n0 + NT],
                                    in_=out_sb[:, n0 : n0 + NT])
    nc.finalize()
    return nc


def _pack_x(xT, dtype, scale=1.0):
    """[512, 2048] fp32 -> [128, NTILES*KC*NT], tile/chunk packed so each
    tile's DMA reads contiguous per-partition lines."""
    a = (xT * scale).reshape(KC, 128, NTILES, NT).transpose(1, 2, 0, 3)
    return np.ascontiguousarray(a.reshape(128, NTILES * KC * NT).astype(dtype))


_NC_CACHE = []


def kernel(movie_vectors, user_vectors, Wm, bm, Wu, bu, W1, b1, W2, b2, W3, b3):
    movie_vectors = np.asarray(movie_vectors, np.float32)
    user_vectors = np.asarray(user_vectors, np.float32)
    wp8, wp = _pack_weights(
        np.asarray(Wm, np.float32), np.asarray(bm, np.float32),
        np.asarray(Wu, np.float32), np.asarray(bu, np.float32),
        np.asarray(W1, np.float32), np.asarray(b1, np.float32),
        np.asarray(W2, np.float32), np.asarray(b2, np.float32),
        np.asarray(W3, np.float32), np.asarray(b3, np.float32),
    )
    xmT = np.ascontiguousarray(movie_vectors.T)  # [512, 16384]
    xuT = np.ascontiguousarray(user_vectors.T)

    if not _NC_CACHE:
        _NC_CACHE.append(_build_bass())
    nc = _NC_CACHE[0]

    in_maps = []
    for c in range(N_CORES):
        sl = slice(c * R, (c + 1) * R)
        in_maps.append(
            {
                "xm8": _pack_x(xmT[:, sl], FP8_NP, SX),
                "xu8": _pack_x(xuT[:, sl], FP8_NP, SX),
                "xm": _pack_x(xmT[:, sl], BF16_NP),
                "xu": _pack_x(xuT[:, sl], BF16_NP),
                "wp8": wp8,
                "wp": wp,
            }
        )
    res = run_bass_kernel_spmd(nc, in_maps, core_ids=list(range(N_CORES)))
    kernel.last_result = res
    return np.concatenate([r["out"].reshape(R, 1) for r in res.results], axis=0)


# revision 14
# speedup vs baseline: 1.1602x; 1.1602x over previous
"""DeepFM forward kernel for Trainium2, data-parallel over 8 NeuronCores.

Math refactor vs the straightforward DeepFM graph:
  sum_ij fm_interactions[b,i,j] = sum_k (sum_i m[b,i,k]) * (sum_j u[b,j,k])
so the BxNMxNU einsum collapses to an 18-dim per-row dot product of "folded"
tower outputs (16 fold products + the two additive terms via const-1 rows).

W1 is folded into the tower weights host-side (Wm1 = Wm_dense @ W1_top,
Wu1 = Wu_dense @ W1_bot) - legal because the first nonlinearity comes after
W1 - so the PE computes h1_pre directly from the inputs.

Everything runs in fp8e4m3 with DoubleRow packing (2 contraction rows per
PE cell, 2x throughput). The logit is dominated by the FM term, which
needs ~bf16 accuracy, so the FM/extras path uses error-compensated fp8:
inputs ship as x8 = fp8(4x) plus a residual r8 = fp8(16*(4x - dec(x8))),
and the extras weights split hi/lo the same way. Three compensation
products (W8'x8 + W8/16'r8 + Wr8'x8) reconstruct the bf16-grade result
while every operand stays fp8. All scale factors fold into host-packed
biases and the final ones-column, so no extra on-chip ops. Total HBM
traffic is 4.65MB/core (x8 + r8 + weights), comfortably below the PE
time, which is the design point: the DMA stream at 6.6MB was the
critical path of the previous revision.
"""

import numpy as np
import ml_dtypes

import concourse.bacc as bacc
import concourse.bass as bass  # noqa: F401
import concourse.mybir as mybir
import concourse.tile as tile
from concourse.bass_utils import run_bass_kernel_spmd

N_CORES = 8
B_FULL = 16384
R = B_FULL // N_CORES  # 2048 rows per core
F = 512                # input features per tower
KC = F // 128          # 4 contraction chunks per tower
NT = 512               # batch tile on the free dim
NTILES = R // NT       # 4
NX = 18                # fold(16) + [add_m, 1] / [1, add_u] rows
N_WARM = 8             # PE pre-warm matmuls (cover the DMA-receipt window)
SW = 16.0              # fp8 tower-weight pre-scale
SX = 4.0               # fp8 input pre-scale
SH = 4.0               # fp8 h1 activation pre-scale
SW2 = 16.0             # fp8 W2 pre-scale
SWE = 8.0              # fp8 extras-weight pre-scale
SR = 16.0              # fp8 input-residual pre-scale
SEX = SWE * SX         # extras PSUM scale (32)

F32 = mybir.dt.float32
BF16 = mybir.dt.bfloat16
FP8 = mybir.dt.float8e4
BF16_NP = ml_dtypes.bfloat16
FP8_NP = ml_dtypes.float8_e4m3
DR = mybir.MatmulPerfMode.DoubleRow

W8COLS = (4 * KC + 2) * 128     # 4 tower groups + W2 double-block
WX8COLS = 12 * 2 * 32           # extras: 2 towers x 3 sets x 2 dchunks

# bf16 weight-pack column offsets ([128, WCOLS] blob)
W3_OFF = 0
ONES_OFF = W3_OFF + 1           # 1
BIAS_OFF = ONES_OFF + 1         # 2
B1A, B1B, B2C, BMX, BUX = range(5)
WCOLS = BIAS_OFF + 5            # 7


def _fold_ext(W, b, is_movie, b3=0.0):
    """[512,257],[257] -> ([512,18], [18]) extras weights/bias."""
    dense_w = W[:, :256]
    fold_w = dense_w.reshape(F, 16, 16).sum(axis=1)        # [512, 16]
    add_w = W[:, 256:257]
    zero_w = np.zeros_like(add_w)
    fold_b = b[:256].reshape(16, 16).sum(axis=0)
    if is_movie:
        tail_w = [add_w, zero_w]
        tail_b = [b[256:257] + b3, np.ones(1, np.float32)]
    else:
        tail_w = [zero_w, add_w]
        tail_b = [np.ones(1, np.float32), b[256:257]]
    w_ext = np.concatenate([fold_w, *tail_w], axis=1)
    b_ext = np.concatenate([fold_b, *tail_b])
    return w_ext.astype(np.float32), b_ext.astype(np.float32)


def _col(vec):
    out = np.zeros((128, 1), np.float32)
    out[: len(vec), 0] = vec
    return out


NXP = 32  # extras M padded to 32: DoubleRow lhsT j-step must be 16B-aligned


def _dr18(Wset):
    """[512, 18] -> [128, 2 dchunks, 2, 32] DoubleRow-packed fp8 block."""
    a = np.zeros((F, NXP), np.float32)
    a[:, :NX] = Wset
    return a.reshape(2, 2, 128, NXP).transpose(2, 0, 1, 3)


def _pack_weights(Wm, bm, Wu, bu, W1, b1, W2, b2, W3, b3):
    b3v = float(np.asarray(b3, np.float32).reshape(1)[0])
    W1t, W1b = W1[:256], W1[256:]
    Wm1 = Wm[:, :256] @ W1t                        # [512, 256] fused tower+W1
    Wu1 = Wu[:, :256] @ W1b
    b1p = b1 + bm[:256] @ W1t + bu[:256] @ W1b     # [256]
    exm_w, exm_b = _fold_ext(Wm, bm, True, b3v)
    exu_w, exu_b = _fold_ext(Wu, bu, False)
    # final reduction column: 1/(SEX^2) per fold-product row
    ones = np.zeros((128, 1), np.float32)
    ones[:NX, 0] = 1.0 / (SEX * SEX)

    # fp8 DoubleRow pack: tower groups [128, g, k, m], then W2 [128, 2, 128]
    w8 = np.zeros((128, 4, KC, 128), np.float32)
    for g, Wg in enumerate((Wm1[:, :128], Wu1[:, :128], Wm1[:, 128:], Wu1[:, 128:])):
        w8[:, g] = (Wg * SW).reshape(KC, 128, 128).transpose(1, 0, 2)
    w2d = (W2.astype(np.float32) * SW2).reshape(2, 128, 128).transpose(1, 0, 2)
    wp8 = np.concatenate(
        [w8.reshape(128, 4 * KC * 128), w2d.reshape(128, 256)], axis=1
    )
    assert wp8.shape == (128, W8COLS)
    wp8 = np.ascontiguousarray(wp8.astype(FP8_NP))

    # extras fp8 hi/lo split: W8 = fp8(W*SWE); Wr8 = fp8(W*SWE - dec(W8));
    # W8div = fp8(W*SWE/SR) pairs with the input residual r8.
    blocks = []
    for Wex in (exm_w, exu_w):
        Whi = (Wex * SWE).astype(FP8_NP).astype(np.float32)
        Wlo = Wex * SWE - Whi
        blocks += [_dr18((Wex * SWE / SR)), _dr18(Whi), _dr18(Wlo)]
    # order per tower: [W8div (r8 pairing), W8, Wr8] - see kernel body
    wx8 = np.stack(blocks).transpose(1, 0, 2, 3, 4)  # [128, 6, 2, 2, 32]
    wx8 = np.ascontiguousarray(wx8.reshape(128, WX8COLS).astype(FP8_NP))

    wp = np.concatenate(
        [
            W3.astype(np.float32).reshape(128, 1),
            ones,
            _col(b1p[:128] * SH), _col(b1p[128:] * SH), _col(b2.astype(np.float32)),
            _col(exm_b * SEX), _col(exu_b * SEX),
        ],
        axis=1,
    )
    assert wp.shape == (128, WCOLS), wp.shape
    return wp8, wx8, wp.astype(BF16_NP)


def _build_bass():
    nc = bacc.Bacc()
    xm8 = nc.dram_tensor("xm8", [128, NTILES * KC * NT], FP8, kind="ExternalInput")
    xu8 = nc.dram_tensor("xu8", [128, NTILES * KC * NT], FP8, kind="ExternalInput")
    xmr = nc.dram_tensor("xmr", [128, NTILES * KC * NT], FP8, kind="ExternalInput")
    xur = nc.dram_tensor("xur", [128, NTILES * KC * NT], FP8, kind="ExternalInput")
    wp8 = nc.dram_tensor("wp8", [128, W8COLS], FP8, kind="ExternalInput")
    wx8d = nc.dram_tensor("wx8", [128, WX8COLS], FP8, kind="ExternalInput")
    wp = nc.dram_tensor("wp", [128, WCOLS], BF16, kind="ExternalInput")
    out = nc.dram_tensor("out", [1, R], F32, kind="ExternalOutput")

    relu = mybir.ActivationFunctionType.Relu
    add = mybir.AluOpType.add
    mult = mybir.AluOpType.mult

    with tile.TileContext(nc) as tc:
        with (
            tc.tile_pool(name="wpool", bufs=1) as wpool,
            tc.tile_pool(name="xpool", bufs=1) as xpool,
            tc.tile_pool(name="dpool", bufs=2) as dpool,
            tc.tile_pool(name="epool", bufs=2) as epool,
            tc.tile_pool(name="opool", bufs=1) as opool,
            tc.tile_pool(name="ps1", bufs=3, space="PSUM") as ps1,
            tc.tile_pool(name="psx", bufs=2, space="PSUM") as psx,
            tc.tile_pool(name="ps2", bufs=1, space="PSUM") as ps2p,
            tc.tile_pool(name="psf", bufs=1, space="PSUM") as psfp,
        ):
            # PE pre-warm on a zeroed tile: keeps the PE busy through the
            # initial DMA+receipt window so real matmuls start warm.
            wgar = wpool.tile([128, NT], BF16)
            nc.vector.memset(wgar, 0.0)
            for _ in range(N_WARM):
                pw = ps1.tile([128, NT], F32, name="ps_mm")
                nc.tensor.matmul(pw, wgar[:, :128], wgar, start=True, stop=True)

            wf8 = wpool.tile([128, 4 * KC + 2, 128], FP8)
            wx8 = wpool.tile([128, 6, 2, 2, 32], FP8)
            w = wpool.tile([128, WCOLS], BF16)
            b = wpool.tile([128, 5], F32)

            nc.scalar.dma_start(
                out=wf8, in_=wp8.rearrange("p (c m) -> p c m", m=128)
            )
            nc.scalar.dma_start(
                out=wx8,
                in_=wx8d.rearrange("p (s d j m) -> p s d j m", s=6, d=2, j=2),
            )
            nc.scalar.dma_start(out=w, in_=wp[:, :])
            nc.vector.tensor_copy(b, w[:, BIAS_OFF:])

            xm8_t = [xpool.tile([128, KC, NT], FP8, name=f"xm8_{t}") for t in range(NTILES)]
            xu8_t = [xpool.tile([128, KC, NT], FP8, name=f"xu8_{t}") for t in range(NTILES)]
            xmr_h = [xpool.tile([128, 2, KC, NT], FP8, name=f"xmrh{h}") for h in range(2)]
            xur_h = [xpool.tile([128, 2, KC, NT], FP8, name=f"xurh{h}") for h in range(2)]
            xm8r = xm8.rearrange("p (t k n) -> p t k n", t=NTILES, k=KC)
            xu8r = xu8.rearrange("p (t k n) -> p t k n", t=NTILES, k=KC)
            xmrr = xmr.rearrange("p (h t k n) -> p h t k n", h=2, t=2, k=KC)
            xurr = xur.rearrange("p (h t k n) -> p h t k n", h=2, t=2, k=KC)
            for h in range(2):
                nc.sync.dma_start(out=xm8_t[2 * h], in_=xm8r[:, 2 * h])
                nc.scalar.dma_start(out=xu8_t[2 * h], in_=xu8r[:, 2 * h])
                nc.sync.dma_start(out=xm8_t[2 * h + 1], in_=xm8r[:, 2 * h + 1])
                nc.scalar.dma_start(out=xu8_t[2 * h + 1], in_=xu8r[:, 2 * h + 1])
                nc.sync.dma_start(out=xmr_h[h], in_=xmrr[:, h])
                nc.scalar.dma_start(out=xur_h[h], in_=xurr[:, h])

            out_sb = opool.tile([1, R], F32)

            for t in range(NTILES):
                n0 = t * NT
                h, tt = divmod(t, 2)
                xm8t, xu8t = xm8_t[t], xu8_t[t]
                xmrt, xurt = xmr_h[h][:, tt], xur_h[h][:, tt]

                # --- fused tower+W1 in fp8 DoubleRow: 2 halves x 4 passes ---
                h1p = dpool.tile([128, 2, NT], FP8, name="h1p")
                for half in range(2):
                    ps = ps1.tile([128, NT], F32, name="ps_mm")
                    for g, xt in ((2 * half, xm8t), (2 * half + 1, xu8t)):
                        for c in range(2):
                            nc.tensor.matmul(
                                ps, wf8[:, g * KC + 2 * c : g * KC + 2 * c + 2, :],
                                xt[:, 2 * c : 2 * c + 2, :],
                                start=(g == 2 * half and c == 0),
                                stop=(g == 2 * half + 1 and c == 1),
                                perf_mode=DR,
                            )
                    # h1' = relu(ps/(SW*SX) + b1)*SH, stored fp8
                    nc.scalar.activation(
                        out=h1p[:, half, :], in_=ps, func=relu,
                        bias=b[:, B1A + half : B1A + half + 1],
                        scale=SH / (SW * SX),
                    )

                # --- tower extras: error-compensated fp8, 6 DR passes each:
                # psx = W8div'r8(x2) + W8'x8(x2) + Wr8'x8(x2), scale SEX ---
                psxm = psx.tile([32, NT], F32, name="ps_x")
                psxu = psx.tile([32, NT], F32, name="ps_x")
                for pstile, tw, x8, xr in ((psxm, 0, xm8t, xmrt), (psxu, 1, xu8t, xurt)):
                    first = True
                    for s, xt in ((0, xr), (1, x8), (2, x8)):
                        for d in range(2):
                            nc.tensor.matmul(
                                pstile, wx8[:, 3 * tw + s, d], xt[:, 2 * d : 2 * d + 2, :],
                                start=first, stop=(s == 2 and d == 1),
                                perf_mode=DR,
                            )
                            first = False
                dmx = epool.tile([NX, NT], BF16, name="dmx")
                nc.vector.tensor_scalar_add(out=dmx, in0=psxm[:NX, :],
                                            scalar1=b[:NX, BMX : BMX + 1])
                prod = epool.tile([NX, NT], BF16, name="prod")
                nc.vector.scalar_tensor_tensor(
                    out=prod, in0=psxu[:NX, :], scalar=b[:NX, BUX : BUX + 1], in1=dmx,
                    op0=add, op1=mult,
                )

                # --- MLP layer 2: one fp8 DoubleRow pass (K=256) ---
                ps2 = ps2p.tile([128, NT], F32, name="ps_l2")
                nc.tensor.matmul(ps2, wf8[:, 4 * KC : 4 * KC + 2, :], h1p,
                                 start=True, stop=True, perf_mode=DR)
                h2 = dpool.tile([128, NT], BF16, name="h2")
                nc.scalar.activation(out=h2, in_=ps2, func=relu,
                                     bias=b[:, B2C : B2C + 1],
                                     scale=1.0 / (SH * SW2))

                # --- final: logit = W3.T @ h2 + (ones/SEX^2).T @ prod ---
                psf = psfp.tile([1, NT], F32, name="ps_fin")
                nc.tensor.matmul(psf, w[:, W3_OFF : W3_OFF + 1], h2,
                                 start=True, stop=False)
                nc.tensor.matmul(psf, w[:NX, ONES_OFF : ONES_OFF + 1], prod,
                                 start=False, stop=True)
                nc.vector.tensor_copy(out_sb[:, n0 : n0 + NT], psf)
                nc.gpsimd.dma_start(out=out[:, n0 : n0 + NT],
                                    in_=out_sb[:, n0 : n0 + NT])
    nc.finalize()
    return nc


def _pack_x(xT, dtype, scale=1.0):
    """[512, 2048] fp32 -> [128, NTILES*KC*NT], tile/chunk packed so each
    tile's DMA reads contiguous per-partition lines."""
    a = (xT * scale).reshape(KC, 128, NTILES, NT).transpose(1, 2, 0, 3)
    return np.ascontiguousarray(a.reshape(128, NTILES * KC * NT).astype(dtype))


_NC_CACHE = []


def kernel(movie_vectors, user_vectors, Wm, bm, Wu, bu, W1, b1, W2, b2, W3, b3):
    movie_vectors = np.asarray(movie_vectors, np.float32)
    user_vectors = np.asarray(user_vectors, np.float32)
    wp8, wx8, wp = _pack_weights(
        np.asarray(Wm, np.float32), np.asarray(bm, np.float32),
        np.asarray(Wu, np.float32), np.asarray(bu, np.float32),
        np.asarray(W1, np.float32), np.asarray(b1, np.float32),
        np.asarray(W2, np.float32), np.asarray(b2, np.float32),
        np.asarray(W3, np.float32), np.asarray(b3, np.float32),
    )
    xmT = np.ascontiguousarray(movie_vectors.T)  # [512, 16384]
    xuT = np.ascontiguousarray(user_vectors.T)

    if not _NC_CACHE:
        _NC_CACHE.append(_build_bass())
    nc = _NC_CACHE[0]

    in_maps = []
    for c in range(N_CORES):
        sl = slice(c * R, (c + 1) * R)
        xm_s = xmT[:, sl] * SX
        xu_s = xuT[:, sl] * SX
        xm8 = xm_s.astype(FP8_NP)
        xu8 = xu_s.astype(FP8_NP)
        xmr = (xm_s - xm8.astype(np.float32)) * SR
        xur = (xu_s - xu8.astype(np.float32)) * SR
        in_maps.append(
            {
                "xm8": _pack_x(xm8.astype(np.float32), FP8_NP),
                "xu8": _pack_x(xu8.astype(np.float32), FP8_NP),
                "xmr": _pack_x(xmr, FP8_NP),
                "xur": _pack_x(xur, FP8_NP),
                "wp8": wp8,
                "wx8": wx8,
                "wp": wp,
            }
        )
    res = run_bass_kernel_spmd(nc, in_maps, core_ids=list(range(N_CORES)))
    kernel.last_result = res
    return np.concatenate([r["out"].reshape(R, 1) for r in res.results], axis=0)
